# revision 31
# baseline (speedup 1.0000x reference)
"""Trainium2 Bass kernel for nn_AttentionResBlock (windowed causal attention +
sigmoid*tanh gating + two 1x1 convs), SPMD over 8 NeuronCores.

Sharding: data-parallel over (batch, sequence-half): core i handles batch i//2,
rows [h*2048, (h+1)*2048) with h = i%2, plus a 512-row halo (previous window;
zeros + mask flag for h==0). No cross-core communication.

Per-core pipeline (window = 512 queries, kv = 1024 keys):
  x (bf16) natural [t,c] tiles via DMA; xT [c,t] via DMA-transpose (16-bit xbar)
  scoresT[j,q] = kvT^T @ qT      (PE bf16, softmax scale folded into exp)
  expT = exp(scale*scoresT)      (ACT, PSUM->SBUF, bf16 out)
  causal mask: affine_select fill=0 on diagonal j-chunks; halo flag multiply
  o_unnorm[q, c+2] = sum_j expT[j,q]^T @ [kv | 1 | 0]  (PE bf16; col 256 =
      softmax denominator, computed by the same matmuls)
  o = o_unnorm[:, :256] * recip(denom)   (DVE per-partition scalar, f32r out)
  oT via PE transpose (f32r) into PSUM; gating reads PSUM directly:
  u = tanh(a) + tanh(a)*tanh(a/2)        (ACT+DVE; the 0.5 from
      sigmoid(a) = (1+tanh(a/2))/2 is folded into the host-side weights)
  res/skip[t,d] = u^T @ (0.5*W^T) + bias (PE f32r + DVE bias-fused copy),
      batched per-window DMA out.

bf16 on the QK/AV path enables fast weight loads (FWL) and halves DMA; the
projection path stays float32r (full-rate fp32) for accuracy. float32r
tensors are storage-compatible with f32 numpy data.
"""

import numpy as np

B, T, C = 4, 4096, 256
W = 512                # attention window
TCH = T // 2           # rows per core
TH = TCH + W           # with halo
NWIN = TCH // W        # windows per core (4)
NCORES = 8

_CACHE = {}


def _build_program(qk_dtype_name="bfloat16"):
    import concourse.bacc as bacc
    import concourse.bass as bass
    import concourse.mybir as mybir
    import concourse.tile as tile
    from concourse.masks import make_identity

    f32 = mybir.dt.float32
    rdt = mybir.dt.float32r
    qdt = getattr(mybir.dt, qk_dtype_name)
    ts = bass.ts

    nc = bacc.Bacc("TRN2", target_bir_lowering=False, debug=False)

    xh = nc.dram_tensor("xh", [TH, C], qdt, kind="ExternalInput").ap()
    wc = nc.dram_tensor("wc", [2, 128, 2 * C], qdt, kind="ExternalInput").ap()
    bb = nc.dram_tensor("bb", [128, 2 * C], f32, kind="ExternalInput").ap()
    hflag = nc.dram_tensor("hflag", [128, 1], f32, kind="ExternalInput").ap()
    res_d = nc.dram_tensor("res", [TCH, C], f32, kind="ExternalOutput").ap()
    skp_d = nc.dram_tensor("skp", [TCH, C], f32, kind="ExternalOutput").ap()

    NBLK = TH // W  # 512-row blocks (5)
    Exp = mybir.ActivationFunctionType.Exp
    Tanh = mybir.ActivationFunctionType.Tanh

    with tile.TileContext(nc) as tc:
        with (
            tc.tile_pool(name="singles", bufs=1) as singles,
            tc.tile_pool(name="xn", bufs=5) as xn_pool,
            tc.tile_pool(name="xt", bufs=5) as xt_pool,
            tc.tile_pool(name="ex", bufs=16) as ex_pool,
            tc.tile_pool(name="on", bufs=6) as on_pool,
            tc.tile_pool(name="g", bufs=3) as g_pool,
            tc.tile_pool(name="outs", bufs=2) as out_pool,
            tc.tile_pool(name="small", bufs=8) as small,
            tc.tile_pool(name="psc", bufs=2, space="PSUM") as sc_pool,
            tc.tile_pool(name="pav", bufs=2, space="PSUM") as avj_pool,
            tc.tile_pool(name="ppj", bufs=2, space="PSUM") as pj_pool,
            tc.tile_pool(name="pt", bufs=2, space="PSUM") as pt_pool,
        ):
            # ---- loads: one HWDGE queue, ordered by when consumers need
            # them. xT transposes for blocks 0,1 gate the first QK; xn 0,1
            # gate the first AV; weights gate the first projection; later
            # blocks stream behind. (Keeping one queue bounds the xbar
            # copy<->transpose mode transitions the HW serializes on.)
            xnb = [None] * NBLK
            xtb = [None] * NBLK
            hf_sb = singles.tile([128, 1], f32)
            wc_sb = singles.tile([128, 2, 2 * C], qdt)
            bb_sb = singles.tile([128, 2 * C], f32)

            def load_xt(blk):
                # split the two xbar transposes across both HWDGE queues
                xt = xt_pool.tile([128, 2, W], qdt, tag="xt")
                for cc, eng in ((0, nc.sync), (1, nc.scalar)):
                    eng.dma_start(
                        out=xt[:, cc, :],
                        in_=xh[ts(blk, W), ts(cc, 128)],
                        transpose=True,
                    )
                xtb[blk] = xt

            def load_xn(blk):
                xn = xn_pool.tile([128, 4, C + 2], qdt, tag="xn")
                nc.sync.dma_start(
                    out=xn[:, :, 0:C],
                    in_=xh[ts(blk, W), :].rearrange("(s p) c -> p s c", p=128),
                )
                xnb[blk] = xn

            identf = singles.tile([128, 128], f32)
            make_identity(nc, identf)
            ident = singles.tile([128, 128], qdt)
            nc.vector.tensor_copy(ident, identf)

            def pe_transpose_xt_cc(blk, cc):
                # PE-path transpose for the startup-critical blocks: real
                # work that also warms the HAM clock-gate during the loads.
                # cc-major emission lets the first QK's cc=0 accumulation
                # half start before the cc=1 transposes finish.
                xn = xnb[blk]
                ptx = pt_pool.tile([128, W], qdt, tag="pt")
                for sub in range(4):
                    nc.tensor.transpose(
                        ptx[:, ts(sub, 128)], xn[:, sub, ts(cc, 128)], ident
                    )
                nc.vector.tensor_copy(xtb[blk][:, cc, :], ptx)

            nc.sync.dma_start(out=hf_sb, in_=hflag)
            load_xn(1)
            load_xn(0)
            nc.scalar.dma_start(out=wc_sb, in_=wc.rearrange("k p n -> p k n"))
            nc.scalar.dma_start(out=bb_sb, in_=bb)
            xt0 = xt_pool.tile([128, 2, W], qdt, tag="xt")
            xt1 = xt_pool.tile([128, 2, W], qdt, tag="xt")
            xtb[0], xtb[1] = xt0, xt1
            for cc in range(2):
                pe_transpose_xt_cc(1, cc)
                pe_transpose_xt_cc(0, cc)
            load_xt(2)
            load_xn(2)
            load_xt(3)
            load_xt(4)
            load_xn(3)
            load_xn(4)
            # [ones, zeros] tail appended to kv tiles (even moving free dim;
            # the ones column yields the softmax denominator inside AV)
            pad2f = singles.tile([128, 4, 2], f32)
            nc.vector.memset(pad2f[:, :, 0:1], 1.0)
            nc.vector.memset(pad2f[:, :, 1:2], 0.0)
            for blk in range(NBLK):
                nc.vector.tensor_copy(xnb[blk][:, :, C : C + 2], pad2f)

            def attn_stage(w):
                """scores -> exp -> mask -> AV -> normalize -> oT (PSUM)."""
                qt = xtb[w + 1]

                # ---- scoresT[j, q] = (kv @ q^T) per j-chunk; exp; mask ----
                # chunks 0..5 full q; 6,7 only q in [256,512), one shared bank
                expts = [None] * 8  # (ap, q_lo) per j-chunk
                for jc in range(6):
                    q_lo = 128 if jc == 5 else 0  # q < 128 fully masked for 5
                    kvt = xtb[w + jc // 4]
                    psc = sc_pool.tile([128, W - q_lo], f32, tag="sc")
                    for cc in range(2):
                        nc.tensor.matmul(
                            psc,
                            kvt[:, cc, ts(jc % 4, 128)],
                            qt[:, cc, q_lo:W],
                            start=(cc == 0),
                            stop=(cc == 1),
                        )
                    ex = ex_pool.tile([128, W - q_lo], qdt, tag="ex2")
                    nc.scalar.activation(out=ex, in_=psc, func=Exp, scale=0.0625)
                    expts[jc] = (ex, q_lo)
                # 6,7 share one PSUM bank as a single accumulation group
                # (start's pending-zero covers the second slice)
                kvt = xtb[w + 1]
                psc = sc_pool.tile([128, 2, 256], f32, tag="sc")
                for i, jc in enumerate((6, 7)):
                    for cc in range(2):
                        nc.tensor.matmul(
                            psc[:, i, :],
                            kvt[:, cc, ts(jc % 4, 128)],
                            qt[:, cc, 256:512],
                            start=(i == 0 and cc == 0),
                            stop=(i == 1 and cc == 1),
                        )
                ex67 = ex_pool.tile([128, 2, 256], qdt, tag="ex1")
                nc.scalar.activation(out=ex67, in_=psc, func=Exp, scale=0.0625)
                expts[6] = (ex67[:, 0, :], 256)
                expts[7] = (ex67[:, 1, :], 256)

                # causal mask: valid iff q - p + 512 - jc*128 >= 0
                for jc in (4, 5):
                    ap, q_lo = expts[jc]
                    nc.gpsimd.affine_select(
                        out=ap,
                        in_=ap,
                        compare_op=mybir.AluOpType.is_ge,
                        fill=0.0,
                        base=q_lo + W - jc * 128,
                        channel_multiplier=-1,
                        pattern=[[1, W - q_lo]],
                    )
                nc.gpsimd.affine_select(
                    out=ex67,
                    in_=ex67,
                    compare_op=mybir.AluOpType.is_ge,
                    fill=0.0,
                    base=0,
                    channel_multiplier=-1,
                    pattern=[[-128, 2], [1, 256]],
                )
                if w == 0:
                    # halo validity flag (1.0 = real halo, 0.0 = first window)
                    for jc in range(4):
                        nc.vector.tensor_scalar_mul(
                            expts[jc][0], expts[jc][0], hf_sb
                        )

                # ---- AV + denom; normalize; transpose to oT (PSUM) ----
                pt4 = pt_pool.tile([128, 2, W], qdt, tag="pt")
                for qb in range(4):
                    jcs = [
                        jc
                        for jc in range(8)
                        if not (qb * 128 + 127) < (jc * 128 - W)
                    ]
                    pav = avj_pool.tile([128, C + 2], f32, tag="av")
                    for k, jc in enumerate(jcs):
                        ap, q_lo = expts[jc]
                        xn = xnb[w + jc // 4]
                        nc.tensor.matmul(
                            pav,
                            ap[:, qb * 128 - q_lo : qb * 128 - q_lo + 128],
                            xn[:, jc % 4, :],
                            start=(k == 0),
                            stop=(k == len(jcs) - 1),
                        )
                    rc = small.tile([128, 1], f32, tag="rc")
                    nc.vector.reciprocal(rc, pav[:, C : C + 1])
                    on = on_pool.tile([128, C], qdt, tag="on")
                    nc.vector.tensor_scalar_mul(on, pav[:, 0:C], rc)
                    for cc in range(2):
                        nc.tensor.transpose(
                            pt4[:, cc, ts(qb, 128)], on[:, ts(cc, 128)], ident
                        )
                return pt4

            def out_stage(w, pt4, last=False):
                """gating -> projections -> bias -> store, for window w."""
                # gating (reads oT straight from PSUM):
                # u = tanh(a) + tanh(a)*tanh(a/2); the 0.5 lives in weights
                th2 = g_pool.tile([128, 2, W], qdt, tag="th2")
                ta = g_pool.tile([128, 2, W], qdt, tag="ta")
                nc.scalar.activation(out=th2, in_=pt4, func=Tanh, scale=0.5)
                nc.scalar.activation(out=ta, in_=pt4, func=Tanh)
                nc.vector.tensor_mul(th2, ta, th2)
                nc.vector.tensor_add(th2, ta, th2)
                us = [th2[:, 0, :], th2[:, 1, :]]

                # projections (res|skip fused along N) + bias; batched store
                rs_win = out_pool.tile([128, 4, 2 * C], f32, tag="rs")
                for qb in range(4):
                    psp = pj_pool.tile([128, 2 * C], f32, tag="pj")
                    for cc in range(2):
                        nc.tensor.matmul(
                            psp,
                            us[cc][:, ts(qb, 128)],
                            wc_sb[:, cc, :],
                            start=(cc == 0),
                            stop=(cc == 1),
                        )
                    nc.vector.tensor_add(rs_win[:, qb, :], psp, bb_sb)
                    if last:
                        # final window: store per q-block so the DMA overlaps
                        # the remaining projections instead of the drain tail
                        trow = w * W + qb * 128
                        nc.sync.dma_start(
                            out=res_d[trow : trow + 128, :],
                            in_=rs_win[:, qb, 0:C],
                        )
                        nc.scalar.dma_start(
                            out=skp_d[trow : trow + 128, :],
                            in_=rs_win[:, qb, C : 2 * C],
                        )
                if not last:
                    nc.sync.dma_start(
                        out=res_d[ts(w, W), :].rearrange("(s p) c -> p s c", p=128),
                        in_=rs_win[:, :, 0:C],
                    )
                    nc.sync.dma_start(
                        out=skp_d[ts(w, W), :].rearrange("(s p) c -> p s c", p=128),
                        in_=rs_win[:, :, C : 2 * C],
                    )

            # software pipeline with a one-window lag: the engine queues are
            # in-order, so window w's projection work is emitted after window
            # w+1's attention stage — the PE runs w+1's QK/AV while ACT/DVE
            # handle w's gating instead of stalling at the window boundary.
            pts = {}
            pts[0] = attn_stage(0)
            for w in range(1, NWIN):
                pts[w] = attn_stage(w)
                out_stage(w - 1, pts.pop(w - 1))
            out_stage(NWIN - 1, pts.pop(NWIN - 1), last=True)

    nc.compile()
    return nc


def _get_program():
    if "nc" not in _CACHE:
        _CACHE["nc"] = _build_program()
    return _CACHE["nc"]


def _make_in_maps(x, Wr, br, Ws, bs):
    import ml_dtypes

    bf16 = ml_dtypes.bfloat16
    x = np.asarray(x, dtype=np.float32)
    Wr = np.asarray(Wr, dtype=np.float32)
    br = np.asarray(br, dtype=np.float32)
    Ws = np.asarray(Ws, dtype=np.float32)
    bs = np.asarray(bs, dtype=np.float32)

    # 0.5x from the sigmoid(a) = (1 + tanh(a/2))/2 identity folded into
    # weights; res and skip projections fused along the output dim
    wcomb = np.concatenate([0.5 * Wr.T, 0.5 * Ws.T], axis=1).reshape(2, 128, 2 * C)
    wcomb = np.ascontiguousarray(wcomb)
    bbt = np.ascontiguousarray(
        np.broadcast_to(np.concatenate([br, bs])[None, :], (128, 2 * C))
    )
    in_maps = []
    for i in range(NCORES):
        b, h = divmod(i, 2)
        xhf = np.empty((TH, C), np.float32)
        if h == 0:
            xhf[:W] = 0.0
            flag = np.zeros((128, 1), np.float32)
        else:
            xhf[:W] = x[b, TCH - W : TCH]
            flag = np.ones((128, 1), np.float32)
        xhf[W:] = x[b, h * TCH : (h + 1) * TCH]
        in_maps.append(
            {
                "xh": np.ascontiguousarray(xhf.astype(bf16)),
                "wc": wcomb.astype(bf16),
                "bb": bbt,
                "hflag": flag,
            }
        )
    return in_maps


def _gather(results):
    residual = np.empty((B, T, C), np.float32)
    skip = np.empty((B, T, C), np.float32)
    for i in range(NCORES):
        b, h = divmod(i, 2)
        residual[b, h * TCH : (h + 1) * TCH] = results[i]["res"]
        skip[b, h * TCH : (h + 1) * TCH] = results[i]["skp"]
    return residual, skip


def kernel(x, Wr, br, Ws, bs):
    from concourse.bass_utils import run_bass_kernel_spmd

    nc = _get_program()
    in_maps = _make_in_maps(x, Wr, br, Ws, bs)
    res = run_bass_kernel_spmd(nc, in_maps, list(range(NCORES)))
    return _gather(res.results)


# revision 32
# speedup vs baseline: 1.1037x; 1.1037x over previous
"""Trainium2 Bass kernel for nn_AttentionResBlock (windowed causal attention +
sigmoid*tanh gating + two 1x1 convs), SPMD over 8 NeuronCores.

Sharding: data-parallel over (batch, sequence-half): core i handles batch i//2,
rows [h*2048, (h+1)*2048) with h = i%2, plus a 512-row halo (previous window;
zeros + mask flag for h==0). No cross-core communication.

Per-core pipeline (window = 512 queries, kv = 1024 keys):
  x (bf16) natural [t,c] tiles via DMA; xT [c,t] via DMA-transpose (16-bit xbar)
  scoresT[j,q] = kvT^T @ qT      (PE bf16, softmax scale folded into exp)
  expT = exp(scale*scoresT)      (ACT, PSUM->SBUF, bf16 out)
  causal mask: affine_select fill=0 on diagonal j-chunks; halo flag multiply
  o_unnorm[q, c+2] = sum_j expT[j,q]^T @ [kv | 1 | 0]  (PE bf16; col 256 =
      softmax denominator, computed by the same matmuls)
  o = o_unnorm[:, :256] * recip(denom)   (DVE per-partition scalar, f32r out)
  oT via PE transpose (f32r) into PSUM; gating reads PSUM directly:
  u = tanh(a) + tanh(a)*tanh(a/2)        (ACT+DVE; the 0.5 from
      sigmoid(a) = (1+tanh(a/2))/2 is folded into the host-side weights)
  res/skip[t,d] = u^T @ (0.5*W^T) + bias (PE f32r + DVE bias-fused copy),
      batched per-window DMA out.

bf16 on the QK/AV path enables fast weight loads (FWL) and halves DMA; the
projection path stays float32r (full-rate fp32) for accuracy. float32r
tensors are storage-compatible with f32 numpy data.
"""

import numpy as np

B, T, C = 4, 4096, 256
W = 512                # attention window
TCH = T // 2           # rows per core
TH = TCH + W           # with halo
NWIN = TCH // W        # windows per core (4)
NCORES = 8

_CACHE = {}


def _build_program(qk_dtype_name="bfloat16"):
    import concourse.bacc as bacc
    import concourse.bass as bass
    import concourse.mybir as mybir
    import concourse.tile as tile
    from concourse.masks import make_identity

    f32 = mybir.dt.float32
    rdt = mybir.dt.float32r
    qdt = getattr(mybir.dt, qk_dtype_name)
    ts = bass.ts

    nc = bacc.Bacc("TRN2", target_bir_lowering=False, debug=False)

    xh = nc.dram_tensor("xh", [TH, C], qdt, kind="ExternalInput").ap()
    wc = nc.dram_tensor("wc", [2, 128, 2 * C], qdt, kind="ExternalInput").ap()
    bb = nc.dram_tensor("bb", [128, 2 * C], f32, kind="ExternalInput").ap()
    hflag = nc.dram_tensor("hflag", [128, 1], f32, kind="ExternalInput").ap()
    res_d = nc.dram_tensor("res", [TCH, C], f32, kind="ExternalOutput").ap()
    skp_d = nc.dram_tensor("skp", [TCH, C], f32, kind="ExternalOutput").ap()

    NBLK = TH // W  # 512-row blocks (5)
    Exp = mybir.ActivationFunctionType.Exp
    Tanh = mybir.ActivationFunctionType.Tanh

    with tile.TileContext(nc) as tc:
        with (
            tc.tile_pool(name="singles", bufs=1) as singles,
            tc.tile_pool(name="xn", bufs=5) as xn_pool,
            tc.tile_pool(name="xt", bufs=5) as xt_pool,
            tc.tile_pool(name="ex", bufs=16) as ex_pool,
            tc.tile_pool(name="on", bufs=6) as on_pool,
            tc.tile_pool(name="g", bufs=3) as g_pool,
            tc.tile_pool(name="outs", bufs=2) as out_pool,
            tc.tile_pool(name="small", bufs=8) as small,
            tc.tile_pool(name="psc", bufs=3, space="PSUM") as sc_pool,
            tc.tile_pool(name="pav", bufs=3, space="PSUM") as avj_pool,
            tc.tile_pool(name="pt", bufs=2, space="PSUM") as pt_pool,
        ):
            # ---- loads: one HWDGE queue, ordered by when consumers need
            # them. xT transposes for blocks 0,1 gate the first QK; xn 0,1
            # gate the first AV; weights gate the first projection; later
            # blocks stream behind. (Keeping one queue bounds the xbar
            # copy<->transpose mode transitions the HW serializes on.)
            xnb = [None] * NBLK
            xtb = [None] * NBLK
            hf_sb = singles.tile([128, 1], f32)
            wc_sb = singles.tile([128, 2, 2 * C], qdt)
            bb_sb = singles.tile([128, 2 * C], f32)

            def load_xt(blk):
                # split the two xbar transposes across both HWDGE queues
                xt = xt_pool.tile([128, 2, W], qdt, tag="xt")
                for cc, eng in ((0, nc.sync), (1, nc.scalar)):
                    eng.dma_start(
                        out=xt[:, cc, :],
                        in_=xh[ts(blk, W), ts(cc, 128)],
                        transpose=True,
                    )
                xtb[blk] = xt

            def load_xn(blk):
                xn = xn_pool.tile([128, 4, C + 2], qdt, tag="xn")
                nc.sync.dma_start(
                    out=xn[:, :, 0:C],
                    in_=xh[ts(blk, W), :].rearrange("(s p) c -> p s c", p=128),
                )
                xnb[blk] = xn

            identf = singles.tile([128, 128], f32)
            make_identity(nc, identf)
            ident = singles.tile([128, 128], qdt)
            nc.vector.tensor_copy(ident, identf)

            def pe_transpose_xt_cc(blk, cc):
                # PE-path transpose for the startup-critical blocks: real
                # work that also warms the HAM clock-gate during the loads.
                # cc-major emission lets the first QK's cc=0 accumulation
                # half start before the cc=1 transposes finish.
                xn = xnb[blk]
                ptx = pt_pool.tile([128, W], qdt, tag="pt")
                for sub in range(4):
                    nc.tensor.transpose(
                        ptx[:, ts(sub, 128)], xn[:, sub, ts(cc, 128)], ident
                    )
                nc.vector.tensor_copy(xtb[blk][:, cc, :], ptx)

            nc.sync.dma_start(out=hf_sb, in_=hflag)
            load_xn(1)
            load_xn(0)
            nc.scalar.dma_start(out=wc_sb, in_=wc.rearrange("k p n -> p k n"))
            nc.scalar.dma_start(out=bb_sb, in_=bb)
            xt0 = xt_pool.tile([128, 2, W], qdt, tag="xt")
            xt1 = xt_pool.tile([128, 2, W], qdt, tag="xt")
            xtb[0], xtb[1] = xt0, xt1
            for cc in range(2):
                pe_transpose_xt_cc(1, cc)
                pe_transpose_xt_cc(0, cc)
            load_xt(2)
            load_xn(2)
            load_xt(3)
            load_xt(4)
            load_xn(3)
            load_xn(4)
            # [ones, zeros] tail appended to kv tiles (even moving free dim;
            # the ones column yields the softmax denominator inside AV)
            pad2f = singles.tile([128, 4, 2], f32)
            nc.vector.memset(pad2f[:, :, 0:1], 1.0)
            nc.vector.memset(pad2f[:, :, 1:2], 0.0)
            for blk in range(NBLK):
                nc.vector.tensor_copy(xnb[blk][:, :, C : C + 2], pad2f)

            def attn_stage(w):
                """scores -> exp -> mask -> AV -> normalize -> oT (PSUM)."""
                qt = xtb[w + 1]

                # ---- scoresT[j, q] = (kv @ q^T) per j-chunk; exp; mask ----
                # chunks 0..5 full q; 6,7 only q in [256,512), one shared bank
                expts = [None] * 8  # (ap, q_lo) per j-chunk
                for jc in range(6):
                    q_lo = 128 if jc == 5 else 0  # q < 128 fully masked for 5
                    kvt = xtb[w + jc // 4]
                    psc = sc_pool.tile([128, W - q_lo], f32, tag="sc")
                    for cc in range(2):
                        nc.tensor.matmul(
                            psc,
                            kvt[:, cc, ts(jc % 4, 128)],
                            qt[:, cc, q_lo:W],
                            start=(cc == 0),
                            stop=(cc == 1),
                        )
                    ex = ex_pool.tile([128, W - q_lo], qdt, tag="ex2")
                    nc.scalar.activation(out=ex, in_=psc, func=Exp, scale=0.0625)
                    expts[jc] = (ex, q_lo)
                # 6,7 share one PSUM bank as a single accumulation group
                # (start's pending-zero covers the second slice)
                kvt = xtb[w + 1]
                psc = sc_pool.tile([128, 2, 256], f32, tag="sc")
                for i, jc in enumerate((6, 7)):
                    for cc in range(2):
                        nc.tensor.matmul(
                            psc[:, i, :],
                            kvt[:, cc, ts(jc % 4, 128)],
                            qt[:, cc, 256:512],
                            start=(i == 0 and cc == 0),
                            stop=(i == 1 and cc == 1),
                        )
                ex67 = ex_pool.tile([128, 2, 256], qdt, tag="ex1")
                nc.scalar.activation(out=ex67, in_=psc, func=Exp, scale=0.0625)
                expts[6] = (ex67[:, 0, :], 256)
                expts[7] = (ex67[:, 1, :], 256)

                # causal mask: valid iff q - p + 512 - jc*128 >= 0
                for jc in (4, 5):
                    ap, q_lo = expts[jc]
                    nc.gpsimd.affine_select(
                        out=ap,
                        in_=ap,
                        compare_op=mybir.AluOpType.is_ge,
                        fill=0.0,
                        base=q_lo + W - jc * 128,
                        channel_multiplier=-1,
                        pattern=[[1, W - q_lo]],
                    )
                nc.gpsimd.affine_select(
                    out=ex67,
                    in_=ex67,
                    compare_op=mybir.AluOpType.is_ge,
                    fill=0.0,
                    base=0,
                    channel_multiplier=-1,
                    pattern=[[-128, 2], [1, 256]],
                )
                if w == 0:
                    # halo validity flag (1.0 = real halo, 0.0 = first window)
                    for jc in range(4):
                        nc.vector.tensor_scalar_mul(
                            expts[jc][0], expts[jc][0], hf_sb
                        )

                # ---- AV + denom; normalize; transpose to oT (PSUM) ----
                pt4 = pt_pool.tile([128, 2, W], qdt, tag="pt")
                for qb in range(4):
                    jcs = [
                        jc
                        for jc in range(8)
                        if not (qb * 128 + 127) < (jc * 128 - W)
                    ]
                    pav = avj_pool.tile([128, C + 2], f32, tag="av")
                    for k, jc in enumerate(jcs):
                        ap, q_lo = expts[jc]
                        xn = xnb[w + jc // 4]
                        nc.tensor.matmul(
                            pav,
                            ap[:, qb * 128 - q_lo : qb * 128 - q_lo + 128],
                            xn[:, jc % 4, :],
                            start=(k == 0),
                            stop=(k == len(jcs) - 1),
                        )
                    rc = small.tile([128, 1], f32, tag="rc")
                    nc.vector.reciprocal(rc, pav[:, C : C + 1])
                    on = on_pool.tile([128, C], qdt, tag="on")
                    nc.vector.tensor_scalar_mul(on, pav[:, 0:C], rc)
                    for cc in range(2):
                        nc.tensor.transpose(
                            pt4[:, cc, ts(qb, 128)], on[:, ts(cc, 128)], ident
                        )
                return pt4

            def out_stage(w, pt4, last=False):
                """gating -> projections -> bias -> store, for window w."""
                # gating (reads oT straight from PSUM):
                # u = tanh(a) + tanh(a)*tanh(a/2); the 0.5 lives in weights
                th2 = g_pool.tile([128, 2, W], qdt, tag="th2")
                ta = g_pool.tile([128, 2, W], qdt, tag="ta")
                nc.scalar.activation(out=th2, in_=pt4, func=Tanh, scale=0.5)
                nc.scalar.activation(out=ta, in_=pt4, func=Tanh)
                nc.vector.tensor_mul(th2, ta, th2)
                nc.vector.tensor_add(th2, ta, th2)
                us = [th2[:, 0, :], th2[:, 1, :]]

                # projections (res|skip fused along N) + bias; batched store
                rs_win = out_pool.tile([128, 4, 2 * C], f32, tag="rs")
                for qb in range(4):
                    psp = avj_pool.tile([128, 2 * C], f32, tag="av")
                    for cc in range(2):
                        nc.tensor.matmul(
                            psp,
                            us[cc][:, ts(qb, 128)],
                            wc_sb[:, cc, :],
                            start=(cc == 0),
                            stop=(cc == 1),
                        )
                    nc.vector.tensor_add(rs_win[:, qb, :], psp, bb_sb)
                    if last:
                        # final window: store per q-block so the DMA overlaps
                        # the remaining projections instead of the drain tail
                        trow = w * W + qb * 128
                        nc.sync.dma_start(
                            out=res_d[trow : trow + 128, :],
                            in_=rs_win[:, qb, 0:C],
                        )
                        nc.scalar.dma_start(
                            out=skp_d[trow : trow + 128, :],
                            in_=rs_win[:, qb, C : 2 * C],
                        )
                if not last:
                    nc.sync.dma_start(
                        out=res_d[ts(w, W), :].rearrange("(s p) c -> p s c", p=128),
                        in_=rs_win[:, :, 0:C],
                    )
                    nc.sync.dma_start(
                        out=skp_d[ts(w, W), :].rearrange("(s p) c -> p s c", p=128),
                        in_=rs_win[:, :, C : 2 * C],
                    )

            # software pipeline with a one-window lag: the engine queues are
            # in-order, so window w's projection work is emitted after window
            # w+1's attention stage — the PE runs w+1's QK/AV while ACT/DVE
            # handle w's gating instead of stalling at the window boundary.
            pts = {}
            pts[0] = attn_stage(0)
            for w in range(1, NWIN):
                pts[w] = attn_stage(w)
                out_stage(w - 1, pts.pop(w - 1))
            out_stage(NWIN - 1, pts.pop(NWIN - 1), last=True)

    nc.compile()
    return nc


def _get_program():
    if "nc" not in _CACHE:
        _CACHE["nc"] = _build_program()
    return _CACHE["nc"]


def _make_in_maps(x, Wr, br, Ws, bs):
    import ml_dtypes

    bf16 = ml_dtypes.bfloat16
    x = np.asarray(x, dtype=np.float32)
    Wr = np.asarray(Wr, dtype=np.float32)
    br = np.asarray(br, dtype=np.float32)
    Ws = np.asarray(Ws, dtype=np.float32)
    bs = np.asarray(bs, dtype=np.float32)

    # 0.5x from the sigmoid(a) = (1 + tanh(a/2))/2 identity folded into
    # weights; res and skip projections fused along the output dim
    wcomb = np.concatenate([0.5 * Wr.T, 0.5 * Ws.T], axis=1).reshape(2, 128, 2 * C)
    wcomb = np.ascontiguousarray(wcomb)
    bbt = np.ascontiguousarray(
        np.broadcast_to(np.concatenate([br, bs])[None, :], (128, 2 * C))
    )
    in_maps = []
    for i in range(NCORES):
        b, h = divmod(i, 2)
        xhf = np.empty((TH, C), np.float32)
        if h == 0:
            xhf[:W] = 0.0
            flag = np.zeros((128, 1), np.float32)
        else:
            xhf[:W] = x[b, TCH - W : TCH]
            flag = np.ones((128, 1), np.float32)
        xhf[W:] = x[b, h * TCH : (h + 1) * TCH]
        in_maps.append(
            {
                "xh": np.ascontiguousarray(xhf.astype(bf16)),
                "wc": wcomb.astype(bf16),
                "bb": bbt,
                "hflag": flag,
            }
        )
    return in_maps


def _gather(results):
    residual = np.empty((B, T, C), np.float32)
    skip = np.empty((B, T, C), np.float32)
    for i in range(NCORES):
        b, h = divmod(i, 2)
        residual[b, h * TCH : (h + 1) * TCH] = results[i]["res"]
        skip[b, h * TCH : (h + 1) * TCH] = results[i]["skp"]
    return residual, skip


def kernel(x, Wr, br, Ws, bs):
    from concourse.bass_utils import run_bass_kernel_spmd

    nc = _get_program()
    in_maps = _make_in_maps(x, Wr, br, Ws, bs)
    res = run_bass_kernel_spmd(nc, in_maps, list(range(NCORES)))
    return _gather(res.results)
